# revision 5
# baseline (speedup 1.0000x reference)
"""Trainium2 Bass kernel for nn_AverageAttention: cumulative-average attention
with a sigmoid gating Linear(2D->2D).

Strategy: data-parallel over batch (B=8 = one batch element per NeuronCore).
All on-chip work happens in transposed space ([feature, token]).
  - cumavg via the affine recurrence avg_t = coef_t*avg_{t-1} + x_t/(t+1),
    one fused tensor_tensor_scan per 512-col chunk on VectorE (host
    pre-scales xdiv = x/(t+1)); chunks chained through a carry tile
  - gating matmul in fp8-e4m3 with perf_mode=DoubleRow: each instruction
    contracts TWO 128-deep k-tiles (weights [128,2,128], moving
    [128,2,512]) at ~2x bf16 throughput (216ns/instr measured). x comes
    pre-cast from host (x8S); avg is cast bf16->fp8 on ScalarE right
    behind each scan. Epilogue precision stays bf16/f32 so only the gate
    pre-activations carry fp8 error (gating rel err ~1.5e-2 < 2e-2).
  - x8/avg8/avg-bf16 live in PER-SLICE SBUF tiles so the tile
    tracker's write-region merging can't invent cross-slice
    dependencies (pass-1 matmuls stalling on slice-2/3 casts)
  - all DRAM layouts are partition-contiguous so HWDGE packets are
    large: W as [DT, P, NK*2P] (one 1MB slab per output tile, streamed
    twice on the sync HWDGE), x8 as [NS, P, KT*TS]; x8/avg8 SBUF tiles
    are slice-major [P, NS, KT, TS] so a whole slice is one DMA and the
    DoubleRow moving AP is [p, 2 (step 512B), 512]
  - startup: runway W streams in 128KB chunks chunk-major across units
    and the 4 runway units' x-half matmuls are emitted k-pair-major
    ACROSS units so the PE starts ~0.6us in and never outruns the W/x8
    streams; coef loads per-slice so scan 0 starts ~1.5us in; slice-0
    xd splits across the gpsimd and scalar queues
  - pass 1 = all 16 output tiles at t-slice 0 (scans+casts for slice 0
    hide behind the runway); scan-sets for slices 1-3 are emitted in
    halves between pass-1 units so no in-order engine stream
    head-blocks an epilogue (psum release); pass 2 is i-outer with the
    4-buf W pool giving multi-unit prefetch across the pass boundary
  - DMA queues: W+bias sync HWDGE; coef/x8/xep/out + slice-0 odd xd
    scalar HWDGE; xd + avg stores gpsimd SWDGE
  - sigmoid+bias fused on ScalarE reading PSUM, combine on VectorE,
    outputs written transposed and un-transposed on host.
"""
import sys

if "/opt/trn_rl_repo" not in sys.path:
    sys.path.insert(0, "/opt/trn_rl_repo")

import numpy as np
import ml_dtypes

B, T, D = 8, 2048, 2048
O = 2 * D          # gate output features (4096)
P = 128            # partitions
KT = D // P        # 16 k-tiles per half of G
DT = D // P        # 16 output-feature tiles
NK = 2 * KT        # 32 k-tiles total
NPAIR = NK // 2    # 16 DoubleRow k-pairs (8 x-pairs + 8 avg-pairs)
TS = 512           # t-slice (matmul moving free dim / scan chunk)
NS = T // TS       # 4 t-slices
RUNWAY = 4         # units whose x-half matmuls front-run the scans

_compiled = None


def _build():
    import concourse.mybir as mybir
    import concourse.tile as tile
    from concourse import bacc

    f32 = mybir.dt.float32
    bf16 = mybir.dt.bfloat16
    f8 = mybir.dt.float8e4
    SIG = mybir.ActivationFunctionType.Sigmoid
    COPY = mybir.ActivationFunctionType.Copy
    DR = mybir.MatmulPerfMode.DoubleRow

    nc = bacc.Bacc(trn_type="TRN2", target_bir_lowering=False, debug=False,
                   num_devices=B)

    xT_d = nc.declare_dram_parameter("xT", [D, T], bf16, isOutput=False)
    x8S_d = nc.declare_dram_parameter("x8S", [NS, P, KT * TS], f8,
                                      isOutput=False)
    xdT_d = nc.declare_dram_parameter("xdT", [D, T], bf16, isOutput=False)
    wP_d = nc.declare_dram_parameter("wP", [DT, P, NK * 2 * P], f8,
                                     isOutput=False)
    wRWx_d = nc.declare_dram_parameter("wRWx", [P, 8, RUNWAY, 2, 2 * P], f8,
                                       isOutput=False)
    wRWa_d = nc.declare_dram_parameter("wRWa", [P, RUNWAY, 8, 2, 2 * P], f8,
                                       isOutput=False)
    bias_d = nc.declare_dram_parameter("bias", [O], f32, isOutput=False)
    coef_d = nc.declare_dram_parameter("coef_t", [1, T], f32, isOutput=False)
    avgT_d = nc.declare_dram_parameter("avgT", [D, T], bf16, isOutput=True)
    outT_d = nc.declare_dram_parameter("outT", [D, T], f32, isOutput=True)

    with tile.TileContext(nc) as tc:
        with tc.tile_pool(name="consts", bufs=1) as consts, \
             tc.tile_pool(name="resid", bufs=1) as resid, \
             tc.tile_pool(name="xdp", bufs=16) as xdp, \
             tc.tile_pool(name="xep", bufs=4) as xep_pool, \
             tc.tile_pool(name="wpool", bufs=5) as wpool, \
             tc.tile_pool(name="avgc", bufs=10) as avgc, \
             tc.tile_pool(name="sigp", bufs=3) as sigp, \
             tc.tile_pool(name="outp", bufs=3) as outp, \
             tc.tile_pool(name="psum", bufs=8, space="PSUM") as pp:

            x8_s = [resid.tile([P, KT, TS], f8, name=f"x8_s{s}")
                    for s in range(NS)]
            avg8_s = [resid.tile([P, KT, TS], f8, name=f"avg8_s{s}")
                      for s in range(NS)]
            wrx = resid.tile([P, 8, RUNWAY, 2, 2 * P], f8)
            wra = resid.tile([P, RUNWAY, 8, 2, 2 * P], f8)

            x8v = x8S_d.rearrange("s p (kt c) -> s p kt c", kt=KT)
            wv = wP_d.rearrange("i p (kt c) -> i p kt c", kt=NK)

            def load_w(i):
                """Two half-tile DMAs on the SAME queue: the x-half
                (kt 0-15) lands first so its matmuls' dependency fires
                at 512KB instead of waiting for the full 1MB tile;
                4KB-contiguous runs keep full packet efficiency."""
                w_i = wpool.tile([P, NK, 2 * P], f8, tag="w", name="w_i")
                nc.sync.dma_start(out=w_i[:, 0:KT, :],
                                  in_=wv[i, :, 0:KT, :])
                nc.sync.dma_start(out=w_i[:, KT:NK, :],
                                  in_=wv[i, :, KT:NK, :])
                return w_i

            # startup streams, earliest consumers first:
            #   sync q:   bias, then runway W chunk-major across units
            #   scalar q: x8 s0 chunks, coef s0
            #   gpsimd q: xd even chunks
            bias_sb = consts.tile([P, O // P], f32)
            nc.sync.dma_start(
                out=bias_sb, in_=bias_d.rearrange("(c p) -> p c", p=P))
            for kp in range(8):
                nc.sync.dma_start(out=wrx[:, kp, :, :, :],
                                  in_=wRWx_d[:, kp, :, :, :])
            for i in range(RUNWAY):
                nc.sync.dma_start(out=wra[:, i, :, :, :],
                                  in_=wRWa_d[:, i, :, :, :])

            coef_sb = consts.tile([P, T], f32)
            nc.scalar.dma_start(out=x8_s[0][:, 0:2, :],
                                in_=x8v[0, :, 0:2, :])
            nc.scalar.dma_start(out=x8_s[0][:, 2:4, :],
                                in_=x8v[0, :, 2:4, :])
            nc.scalar.dma_start(out=coef_sb[:, 0:TS],
                                in_=coef_d[:, 0:TS].to_broadcast((P, TS)))
            for a in range(4, KT, 2):
                nc.scalar.dma_start(out=x8_s[0][:, a:a + 2, :],
                                    in_=x8v[0, :, a:a + 2, :])

            carry = consts.tile([P, KT], f32)

            def load_x8(s):
                nc.scalar.dma_start(out=x8_s[s][:, :, :],
                                    in_=x8v[s, :, :, :])

            def load_coef(s):
                sl = slice(s * TS, (s + 1) * TS)
                nc.scalar.dma_start(
                    out=coef_sb[:, sl],
                    in_=coef_d[:, sl].to_broadcast((P, TS)))

            def scan_set(s, jlo=0, jhi=KT, split_q=False):
                """Phase-A ops for chunks [jlo,jhi) of slice s. xd loads
                are issued up-front so the in-order gpsimd stream never
                blocks a later xd issue behind a scan-dependent write;
                the fp8 cast rides ScalarE right behind each scan."""
                sl = slice(s * TS, (s + 1) * TS)
                xds = []
                for j in range(jlo, jhi):
                    rows = slice(j * P, (j + 1) * P)
                    xd = xdp.tile([P, TS], bf16, tag="xd", name="xd")
                    eng = nc.scalar if (split_q and j % 2 == 1) else nc.gpsimd
                    eng.dma_start(out=xd, in_=xdT_d[rows, sl])
                    xds.append(xd)
                for j in range(jlo, jhi):
                    rows = slice(j * P, (j + 1) * P)
                    avc = avgc.tile([P, TS], bf16, tag="avc", name="avc")
                    nc.vector.tensor_tensor_scan(
                        out=avc, data0=coef_sb[:, sl], data1=xds[j - jlo],
                        initial=(0.0 if s == 0 else carry[:, j:j + 1]),
                        op0=mybir.AluOpType.mult, op1=mybir.AluOpType.add)
                    if s < NS - 1:
                        nc.vector.tensor_copy(carry[:, j:j + 1],
                                              avc[:, TS - 1:TS])
                    nc.scalar.activation(avg8_s[s][:, j, :], avc, COPY)
                    nc.gpsimd.dma_start(out=avgT_d[rows, sl], in_=avc)

            def rhs_for(kp, s):
                if kp < NPAIR // 2:
                    return x8_s[s][:, 2 * kp:2 * kp + 2, :]
                jj = 2 * (kp - NPAIR // 2)
                return avg8_s[s][:, jj:jj + 2, :]

            def mm_half(ps_ig, ps_fg, w_i, s, half):
                kps = range(0, NPAIR // 2) if half == 0 \
                    else range(NPAIR // 2, NPAIR)
                for kp in kps:
                    nc.tensor.matmul(ps_ig, lhsT=w_i[:, 2 * kp:2 * kp + 2, 0:P],
                                     rhs=rhs_for(kp, s), start=(kp == 0),
                                     stop=(kp == NPAIR - 1), perf_mode=DR)
                for kp in kps:
                    nc.tensor.matmul(ps_fg,
                                     lhsT=w_i[:, 2 * kp:2 * kp + 2, P:2 * P],
                                     rhs=rhs_for(kp, s), start=(kp == 0),
                                     stop=(kp == NPAIR - 1), perf_mode=DR)

            def epilogue(ps_ig, ps_fg, i, s):
                sl = slice(s * TS, (s + 1) * TS)
                x_ep = xep_pool.tile([P, TS], bf16, tag="xe", name="x_ep")
                nc.scalar.dma_start(out=x_ep,
                                    in_=xT_d[i * P:(i + 1) * P, sl])
                sig_i = sigp.tile([P, TS], f32, tag="sig", name="sig_i")
                nc.scalar.activation(sig_i, ps_ig, SIG,
                                     bias=bias_sb[:, i:i + 1])
                sig_f = sigp.tile([P, TS], f32, tag="sig", name="sig_f")
                nc.scalar.activation(sig_f, ps_fg, SIG,
                                     bias=bias_sb[:, KT + i:KT + i + 1])
                out_s = outp.tile([P, TS], f32, tag="out", name="out_s")
                nc.vector.tensor_mul(out_s, sig_i, x_ep)
                nc.vector.tensor_mul(sig_f, sig_f, avg8_s[s][:, i, :])
                nc.vector.tensor_add(out_s, out_s, sig_f)
                nc.scalar.dma_start(out=outT_d[i * P:(i + 1) * P, sl],
                                     in_=out_s)

            def full_unit(w_i, i, s):
                ps_ig = pp.tile([P, TS], f32, tag="ps", name="ps_ig")
                ps_fg = pp.tile([P, TS], f32, tag="ps", name="ps_fg")
                mm_half(ps_ig, ps_fg, w_i, s, half=0)
                mm_half(ps_ig, ps_fg, w_i, s, half=1)
                epilogue(ps_ig, ps_fg, i, s)

            # ---- pass 1 (s = 0 across all i): runway x-halves emitted
            # ---- k-pair-major ACROSS the 4 units (trickles in behind
            # ---- the chunked W/x8 streams), then per-unit
            # ---- avg-half+epilogue; scan-sets for slices 1-3 spread in
            # ---- halves between units
            scan_set(0, split_q=True)
            load_coef(1)
            run_ps = [(pp.tile([P, TS], f32, tag="ps", name="ps_rw_i"),
                       pp.tile([P, TS], f32, tag="ps", name="ps_rw_f"))
                      for _ in range(RUNWAY)]
            for kp in range(NPAIR // 2):
                for i in range(RUNWAY):
                    ps_ig, ps_fg = run_ps[i]
                    nc.tensor.matmul(
                        ps_ig, lhsT=wrx[:, kp, i, :, 0:P],
                        rhs=rhs_for(kp, 0), start=(kp == 0),
                        stop=False, perf_mode=DR)
                    nc.tensor.matmul(
                        ps_fg, lhsT=wrx[:, kp, i, :, P:2 * P],
                        rhs=rhs_for(kp, 0), start=(kp == 0),
                        stop=False, perf_mode=DR)
            for i in range(RUNWAY):
                ps_ig, ps_fg = run_ps[i]
                for q in range(8):
                    nc.tensor.matmul(
                        ps_ig, lhsT=wra[:, i, q, :, 0:P],
                        rhs=rhs_for(8 + q, 0), start=False,
                        stop=(q == 7), perf_mode=DR)
                for q in range(8):
                    nc.tensor.matmul(
                        ps_fg, lhsT=wra[:, i, q, :, P:2 * P],
                        rhs=rhs_for(8 + q, 0), start=False,
                        stop=(q == 7), perf_mode=DR)
                epilogue(ps_ig, ps_fg, i, 0)
                if i == 1:
                    scan_set(1, 0, 8)
                    load_x8(1)
                elif i == 3:
                    scan_set(1, 8, KT)
                    load_coef(2)
            # ---- pass A tail: units 4-15 run slices 0 AND 1 off one W
            # ---- load, halving the pass-A W stream rate (16MB over ~200us
            # ---- instead of ~125us); scan-sets for slices 2-3 spread
            # ---- between units
            for i in range(RUNWAY, DT):
                w_i = load_w(i)
                full_unit(w_i, i, 0)
                if i == 8:
                    scan_set(2, 0, 8)
                    load_x8(2)
                elif i == 10:
                    scan_set(2, 8, KT)
                    load_coef(3)
                elif i == 12:
                    scan_set(3, 0, 8)
                    load_x8(3)
                elif i == 14:
                    scan_set(3, 8, KT)
                full_unit(w_i, i, 1)

            # ---- pass B: runway units catch up on slice 1, then
            # ---- everyone's slices 2-3 (W reloaded once; the 4-buf
            # ---- pool prefetches across the pass boundary) ----
            for i in range(DT):
                w_i = load_w(i)
                for s in ((1, 2, 3) if i < RUNWAY else (2, 3)):
                    full_unit(w_i, i, s)

    nc.compile()
    return nc


def _get_compiled():
    global _compiled
    if _compiled is None:
        _compiled = _build()
    return _compiled


def _run(inputs, trace=False, **spmd_kwargs):
    from concourse.bass_utils import run_bass_kernel_spmd

    nc = _get_compiled()
    layer_in = np.asarray(inputs["layer_in"], dtype=np.float32)
    W_gate = np.asarray(inputs["W_gate"], dtype=np.float32)
    b_gate = np.asarray(inputs["b_gate"], dtype=np.float32)

    # wP[i, p, kt*256 + c] = W^T[kt*128 + p, gate-tile i column c]
    # (c < 128: input-gate cols i*128+c; c >= 128: forget-gate cols
    #  D + i*128 + (c-128)) — partition-contiguous 8KB rows per unit
    wT = np.ascontiguousarray(W_gate.T)                    # [k, o]
    wP = np.ascontiguousarray(
        wT.reshape(NK, P, 2, DT, P).transpose(3, 1, 0, 2, 4)
        .reshape(DT, P, NK * 2 * P)
    ).astype(ml_dtypes.float8_e4m3)
    # runway W packed in exact consumption order: x-halves k-pair-major
    # across units [p, kp, i, two, c], avg-halves unit-major [p, i, q, two, c]
    arr = np.asarray(wP[:RUNWAY]).reshape(RUNWAY, P, NK, 2 * P)
    wRWx = np.ascontiguousarray(
        arr[:, :, 0:KT, :].reshape(RUNWAY, P, 8, 2, 2 * P)
        .transpose(1, 2, 0, 3, 4))
    wRWa = np.ascontiguousarray(
        arr[:, :, KT:NK, :].reshape(RUNWAY, P, 8, 2, 2 * P)
        .transpose(1, 0, 2, 3, 4))
    tt = np.arange(T, dtype=np.float32)
    coef = (tt / (tt + 1.0)).reshape(1, T)
    inv = (1.0 / (tt + 1.0)).reshape(1, T)

    in_maps = []
    for b in range(B):
        xTb = np.ascontiguousarray(layer_in[b].T)
        # x8S[s, p, j*TS + c] = fp8(x^T[j*128 + p, s*512 + c])
        x8S = np.ascontiguousarray(
            xTb.reshape(KT, P, NS, TS).transpose(2, 1, 0, 3)
            .reshape(NS, P, KT * TS)
        ).astype(ml_dtypes.float8_e4m3)
        in_maps.append({
            "xT": xTb.astype(ml_dtypes.bfloat16),
            "x8S": x8S,
            "xdT": (xTb * inv).astype(ml_dtypes.bfloat16),
            "wP": wP,
            "wRWx": wRWx,
            "wRWa": wRWa,
            "bias": b_gate,
            "coef_t": coef,
        })

    res = run_bass_kernel_spmd(nc, in_maps, core_ids=list(range(B)),
                               trace=trace, **spmd_kwargs)
    gating = np.empty((B, T, D), dtype=np.float32)
    avg = np.empty((B, T, D), dtype=np.float32)
    for b in range(B):
        gating[b] = res.results[b]["outT"].T
        avg[b] = res.results[b]["avgT"].astype(np.float32).T
    return (gating, avg), res


def kernel(**inputs):
    (gating, avg), _ = _run(inputs, trace=False)
    return gating, avg
